# revision 1
# baseline (speedup 1.0000x reference)
"""MFDWC feature extractor as a Bass/Tile kernel for TRN2 (8 NeuronCores).

Pipeline (per batch row): pre-emphasis -> framing (999 frames x 882 samples,
hop 441) -> Hamming window -> rFFT(2048) power spectrum -> mel (60) -> log ->
Haar DWT -> delta -> mean/std over time -> 180 features.

Device mapping:
  - Data parallel: 16 batch rows -> 2 rows per core on 8 cores.
  - rFFT is computed as two DFT matmuls (cos / sin matrices, window folded in)
    in fp16 at full PE rate, fp32 PSUM accumulation.
  - The waveform is reshaped on-chip to put the sample-within-frame axis on
    SBUF partitions (PE transposes of 441-wide chunks); frames then appear as
    overlapping column views of a single (441, 1000) buffer.
  - Bins packing: cos matmul covers bins 0..1023; the sin matrix's bin-0
    column (which would be all zeros) instead carries the Nyquist cos column,
    and the two mel matrices are adjusted to match, so all 1025 power bins are
    covered by 2x1024 columns with no waste.
  - power -> mel is a second (tiny) matmul pair; log/Haar/delta/stats run on
    ACT/DVE engines.
"""

import math
from contextlib import ExitStack

import numpy as np

import concourse.bass as bass
import concourse.bacc as bacc
import concourse.mybir as mybir
import concourse.tile as tile
from concourse.bass_utils import run_bass_kernel_spmd

F32 = mybir.dt.float32
F16 = mybir.dt.float16
AF = mybir.ActivationFunctionType

B = 16               # batch
L = 441000           # samples per row
W = 441              # hop; also chunk width
NK = 1000            # number of 441-sample chunks per row (441*1000 = L)
FRAME = 882          # frame length
T = 999              # frames per row
NB = 1024            # matmul bins (bins 0..1023; Nyquist packed into sin col 0)
NMEL = 60
ROWS = 2             # batch rows per core
EPS = 1e-10
SQRT2 = math.sqrt(2.0)

# contraction chunks over the 882 frame samples: (r0, size, a) where the
# frame-sample index j = 441*a + r0 + i
KCH = [(0, 128, 0), (128, 128, 0), (256, 128, 0), (384, 57, 0),
       (0, 128, 1), (128, 128, 1), (256, 128, 1), (384, 57, 1)]
# chunks over the NK=1000 waveform rows
ECH = [(k * 128, min(128, NK - k * 128)) for k in range(8)]
# transpose row-blocks over the 441 samples per chunk
RBL = [(0, 128), (128, 128), (256, 128), (384, 57)]
# frame chunks (PSUM free-dim <= 512 fp32)
FCH = [(0, 512), (512, 487)]


def _host_constants(mel_filters: np.ndarray):
    """DFT / mel matrices with window folded in (fp16)."""
    j = np.arange(FRAME, dtype=np.float64)
    b = np.arange(NB, dtype=np.float64)
    ham = np.hamming(FRAME).astype(np.float64)
    ang = 2.0 * np.pi * np.outer(j, b) / 2048.0
    cw = (ham[:, None] * np.cos(ang)).astype(np.float16)          # (882, 1024)
    sw = ham[:, None] * np.sin(ang)
    sw[:, 0] = ham * np.cos(np.pi * j)                            # Nyquist cos col
    sw = sw.astype(np.float16)                                    # (882, 1024)
    m = mel_filters.astype(np.float64)                            # (60, 1025)
    mat = m[:, 0:NB].T.astype(np.float16)                         # (1024, 60)
    mbt = np.concatenate([m[:, NB:NB + 1], m[:, 1:NB]], axis=1).T.astype(np.float16)
    idn = np.eye(128, dtype=np.float16)
    hsum = np.zeros((NMEL, 30), np.float16)
    hdif = np.zeros((NMEL, 30), np.float16)
    for i in range(30):
        hsum[2 * i, i] = 1.0
        hsum[2 * i + 1, i] = 1.0
        hdif[2 * i, i] = 1.0
        hdif[2 * i + 1, i] = -1.0
    return cw, sw, mat, mbt, idn, hsum, hdif


def _body(ctx: ExitStack, tc, xpad, cw_d, sw_d, mat_d, mbt_d, idn_d, hs_d, hd_d, out_d):
    nc = tc.nc

    const = ctx.enter_context(tc.tile_pool(name="const", bufs=1))
    e2p = ctx.enter_context(tc.tile_pool(name="e2", bufs=3))
    emphp = ctx.enter_context(tc.tile_pool(name="emph", bufs=3))
    etp = ctx.enter_context(tc.tile_pool(name="et", bufs=1))
    ptrp = ctx.enter_context(tc.tile_pool(name="ptr", bufs=1, space="PSUM"))
    dftp = ctx.enter_context(tc.tile_pool(name="dft", bufs=2, space="PSUM"))
    melp = ctx.enter_context(tc.tile_pool(name="mel", bufs=1, space="PSUM"))
    haarp = ctx.enter_context(tc.tile_pool(name="haar", bufs=1, space="PSUM"))
    ppp = ctx.enter_context(tc.tile_pool(name="pp", bufs=2))
    lmp = ctx.enter_context(tc.tile_pool(name="lm", bufs=1))
    hop = ctx.enter_context(tc.tile_pool(name="ho", bufs=1))
    stp = ctx.enter_context(tc.tile_pool(name="st", bufs=2))

    # constants
    cw_t, sw_t = [], []
    for ki, (r0, sz, a) in enumerate(KCH):
        j0 = 441 * a + r0
        t = const.tile([128, NB], F16, tag=f"cw{ki}", name=f"cw{ki}")
        nc.sync.dma_start(t[0:sz, :], cw_d[j0:j0 + sz, :])
        cw_t.append(t)
        t = const.tile([128, NB], F16, tag=f"sw{ki}", name=f"sw{ki}")
        nc.sync.dma_start(t[0:sz, :], sw_d[j0:j0 + sz, :])
        sw_t.append(t)
    mat_t, mbt_t = [], []
    for c in range(8):
        t = const.tile([128, NMEL], F16, tag=f"ma{c}", name=f"ma{c}")
        nc.sync.dma_start(t[:, :], mat_d[c * 128:(c + 1) * 128, :])
        mat_t.append(t)
        t = const.tile([128, NMEL], F16, tag=f"mb{c}", name=f"mb{c}")
        nc.sync.dma_start(t[:, :], mbt_d[c * 128:(c + 1) * 128, :])
        mbt_t.append(t)
    ident = const.tile([128, 128], F16, tag="id", name="ident")
    nc.sync.dma_start(ident[:, :], idn_d[:, :])
    eps_t = const.tile([128, 1], F32, tag="eps", name="eps")
    nc.vector.memset(eps_t[:, :], EPS)
    hs_t = const.tile([NMEL, 30], F16, tag="hs", name="hs")
    nc.sync.dma_start(hs_t[:, :], hs_d[:, :])
    hd_t = const.tile([NMEL, 30], F16, tag="hd", name="hd")
    nc.sync.dma_start(hd_t[:, :], hd_d[:, :])

    for r in range(ROWS):
        # ---- phase 1: pre-emphasis + on-chip transpose to (441, 1000) fp16
        et = [etp.tile([128, NK], F16, tag=f"et{r}_{c}", name=f"et{r}_{c}") for c in range(4)]
        for (k0, ksz) in ECH:
            e2 = e2p.tile([128, W + 1], F32, tag="e2", name="e2")
            src = bass.AP(xpad, r * (L + 1) + W * k0, [[W, ksz], [1, W + 1]])
            nc.sync.dma_start(e2[0:ksz, :], src)
            tmp = emphp.tile([128, W], F16, tag="tmp", name="tmp")
            nc.scalar.mul(tmp[0:ksz, :], e2[0:ksz, 0:W], 0.97)
            em = emphp.tile([128, W], F16, tag="em", name="em")
            nc.vector.tensor_sub(em[0:ksz, :], e2[0:ksz, 1:W + 1], tmp[0:ksz, :])
            for rb, (rb0, rbsz) in enumerate(RBL):
                ptr = ptrp.tile([128, 128], F16, tag="ptr", name="ptr")
                nc.tensor.transpose(ptr[0:rbsz, 0:ksz], em[0:ksz, rb0:rb0 + rbsz],
                                    ident[0:ksz, 0:ksz])
                nc.scalar.copy(et[rb][0:rbsz, k0:k0 + ksz], ptr[0:rbsz, 0:ksz])

        # ---- phase 2: DFT power -> mel -> log
        lm = lmp.tile([NMEL, T], F16, tag=f"lm{r}", name=f"lm{r}")
        for (f0, fN) in FCH:
            mp = melp.tile([NMEL, 512], F32, tag="mp", name="mp")
            for bc in range(8):
                pre = dftp.tile([128, 512], F32, tag="pre", name="pre")
                pim = dftp.tile([128, 512], F32, tag="pim", name="pim")
                for ki, (r0, sz, a) in enumerate(KCH):
                    rhs = et[r0 // 128][0:sz, f0 + a:f0 + a + fN]
                    nc.tensor.matmul(pre[:, 0:fN], cw_t[ki][0:sz, bc * 128:(bc + 1) * 128],
                                     rhs, start=(ki == 0), stop=(ki == 7))
                for ki, (r0, sz, a) in enumerate(KCH):
                    rhs = et[r0 // 128][0:sz, f0 + a:f0 + a + fN]
                    nc.tensor.matmul(pim[:, 0:fN], sw_t[ki][0:sz, bc * 128:(bc + 1) * 128],
                                     rhs, start=(ki == 0), stop=(ki == 7))
                pa = ppp.tile([128, 512], F16, tag="pa", name="pa")
                nc.scalar.square(pa[:, 0:fN], pre[:, 0:fN])
                pb = ppp.tile([128, 512], F16, tag="pb", name="pb")
                nc.scalar.square(pb[:, 0:fN], pim[:, 0:fN])
                nc.tensor.matmul(mp[0:NMEL, 0:fN], mat_t[bc][:, 0:NMEL], pa[:, 0:fN],
                                 start=(bc == 0), stop=False, skip_group_check=True)
                nc.tensor.matmul(mp[0:NMEL, 0:fN], mbt_t[bc][:, 0:NMEL], pb[:, 0:fN],
                                 start=False, stop=(bc == 7), skip_group_check=True)
            nc.scalar.activation(lm[0:NMEL, f0:f0 + fN], mp[0:NMEL, 0:fN], AF.Ln,
                                 bias=eps_t[0:NMEL, :])

        # ---- phase 3: Haar (as tiny matmuls) / delta / stats
        ca = hop.tile([30, T], F32, tag=f"ca{r}", name=f"ca{r}")
        cd = hop.tile([30, T], F32, tag=f"cd{r}", name=f"cd{r}")
        for (f0, fN) in FCH:
            pca = haarp.tile([30, 512], F32, tag="pca", name="pca")
            nc.tensor.matmul(pca[:, 0:fN], hs_t[:, :], lm[0:NMEL, f0:f0 + fN],
                             start=True, stop=True, skip_group_check=True)
            nc.scalar.copy(ca[:, f0:f0 + fN], pca[:, 0:fN])
            pcd = haarp.tile([30, 512], F32, tag="pcd", name="pcd")
            nc.tensor.matmul(pcd[:, 0:fN], hd_t[:, :], lm[0:NMEL, f0:f0 + fN],
                             start=True, stop=True, skip_group_check=True)
            nc.scalar.copy(cd[:, f0:f0 + fN], pcd[:, 0:fN])
        dl = hop.tile([30, T], F32, tag=f"dl{r}", name=f"dl{r}")
        nc.vector.tensor_sub(dl[:, 1:T - 1], ca[:, 2:T], ca[:, 0:T - 2])
        nc.vector.tensor_sub(dl[:, 0:1], ca[:, 1:2], ca[:, 0:1])
        nc.vector.tensor_sub(dl[:, T - 1:T], ca[:, T - 1:T], ca[:, T - 2:T - 1])

        stats = stp.tile([30, 6], F32, tag=f"stats{r}", name=f"stats{r}")
        for si, feat in enumerate((ca, dl, cd)):
            s1 = stp.tile([30, 1], F32, tag="s1", name="s1")
            nc.vector.tensor_reduce(s1[:, :], feat[:, :], axis=mybir.AxisListType.X,
                                    op=mybir.AluOpType.add)
            nc.vector.tensor_scalar_mul(stats[:, si:si + 1], s1[:, :], 1.0 / (T * SQRT2))
            nm = stp.tile([30, 1], F32, tag="nm", name="nm")
            nc.vector.tensor_scalar_mul(nm[:, :], s1[:, :], -1.0 / T)
            scr = stp.tile([30, T], F32, tag="scr", name="scr")
            sq = stp.tile([30, 1], F32, tag="sq", name="sq")
            nc.scalar.activation(scr[:, :], feat[:, :], AF.Square, bias=nm[:, :],
                                 scale=1.0, accum_out=sq[:, :])
            nc.scalar.activation(stats[:, 3 + si:4 + si], sq[:, :], AF.Sqrt,
                                 scale=1.0 / ((T - 1) * 2.0))
        nc.sync.dma_start(bass.AP(out_d, r * 180, [[1, 180]]), stats[:, :])


_CACHE = {}


def _build():
    if "nc" in _CACHE:
        return _CACHE["nc"]
    nc = bacc.Bacc("TRN2", target_bir_lowering=False, debug=False,
                   enable_asserts=False, num_devices=8)
    xpad = nc.dram_tensor("xpad", [ROWS, L + 1], F32, kind="ExternalInput")
    cw_d = nc.dram_tensor("cw", [FRAME, NB], F16, kind="ExternalInput")
    sw_d = nc.dram_tensor("sw", [FRAME, NB], F16, kind="ExternalInput")
    mat_d = nc.dram_tensor("mat", [NB, NMEL], F16, kind="ExternalInput")
    mbt_d = nc.dram_tensor("mbt", [NB, NMEL], F16, kind="ExternalInput")
    idn_d = nc.dram_tensor("idn", [128, 128], F16, kind="ExternalInput")
    hs_d = nc.dram_tensor("hsum", [NMEL, 30], F16, kind="ExternalInput")
    hd_d = nc.dram_tensor("hdif", [NMEL, 30], F16, kind="ExternalInput")
    out_d = nc.dram_tensor("out", [ROWS, 180], F32, kind="ExternalOutput")
    with tile.TileContext(nc) as tc, ExitStack() as ctx:
        _body(ctx, tc, xpad, cw_d, sw_d, mat_d, mbt_d, idn_d, hs_d, hd_d, out_d)
    nc.compile()
    _CACHE["nc"] = nc
    return nc


def make_in_maps(waveform: np.ndarray, mel_filters: np.ndarray):
    cw, sw, mat, mbt, idn, hsum, hdif = _host_constants(mel_filters)
    in_maps = []
    for core in range(8):
        rows = waveform[ROWS * core:ROWS * (core + 1)]
        xpad = np.zeros((ROWS, L + 1), np.float32)
        xpad[:, 1:] = rows
        in_maps.append({"xpad": xpad, "cw": cw, "sw": sw, "mat": mat,
                        "mbt": mbt, "idn": idn, "hsum": hsum, "hdif": hdif})
    return in_maps


def gather_out(results):
    # device rows are packed [mel_idx, stat]; reorder to [stat, mel_idx]
    full = np.concatenate([results[c]["out"] for c in range(8)], axis=0)
    return np.ascontiguousarray(
        full.reshape(B, 30, 6).transpose(0, 2, 1).reshape(B, 180)).astype(np.float32)


def run(waveform, mel_filters, trace=False):
    nc = _build()
    in_maps = make_in_maps(np.asarray(waveform, np.float32),
                           np.asarray(mel_filters, np.float32))
    res = run_bass_kernel_spmd(nc, in_maps, core_ids=list(range(8)), trace=trace)
    return gather_out(res.results), res


def kernel(waveform: np.ndarray, mel_filters: np.ndarray) -> np.ndarray:
    out, _ = run(waveform, mel_filters, trace=False)
    return out



# revision 9
# speedup vs baseline: 1.5348x; 1.5348x over previous
"""MFDWC feature extractor as a Bass/Tile kernel for TRN2 (8 NeuronCores).

v2: fp8 DoubleRow DFT with pre-emphasis folded into the DFT matrices.

Pipeline (per batch row): [pre-emphasis folded into weights] -> framing
(999 frames x 883 samples of the zero-padded raw waveform, hop 441) ->
rFFT(2048) power spectrum -> mel (60) -> log -> Haar DWT -> delta ->
mean/std over time -> 180 features.

Device mapping / math:
  - Data parallel: 16 batch rows -> 2 rows per core on 8 cores.
  - Pre-emphasis y[t] = x[t] - 0.97 x[t-1] is an LTI op, so it is folded
    into the DFT coefficient matrix: frame DFT = sum_m xpad[441 t + m] *
    C[m, k] over m in [0, 883), with C[m,k] = w[m-1] e^{-i th (m-1)} -
    0.97 w[m] e^{-i th m}. The kernel contracts 883 raw samples per frame.
  - The waveform is transposed on-chip to (441-sample chunk rows x 1000
    chunk cols) fp8 tiles; frames are overlapping column views.
  - DoubleRow fp8 matmuls contract 256 rows per instruction: pair o=0 ->
    sample j = 128 q + p (column c), o=1 -> j = 441 + 128 q + p (column
    c+1, expressed as an overlapping access pattern on the same tile).
    Tail rows j in [384,441) u [825,883) live in one combined tile
    (partitions 0..56 and 64..121, the latter written by a base-64
    tile_position transpose) contracted by one normal fp8 matmul.
  - Bins packing: cos matmul covers bins 0..1023; the sin matrix's bin-0
    column carries the Nyquist cos column; mel matrices match.
  - power -> mel is one DoubleRow matmul per 128-bin block (cos^2 block
    paired with sin^2 block); power is stored fp8 scaled by 1/256, the
    log offset is repaid in the cA-mean statistics.
"""

import math
from contextlib import ExitStack

import numpy as np

import concourse.bass as bass
import concourse.bacc as bacc
import concourse.mybir as mybir
import concourse.tile as tile
from concourse.bass_utils import run_bass_kernel_spmd

F32 = mybir.dt.float32
F16 = mybir.dt.float16
F8 = mybir.dt.float8e4
AF = mybir.ActivationFunctionType
DR = mybir.MatmulPerfMode.DoubleRow

B = 16               # batch
L = 441000           # samples per row
W = 441              # hop; also chunk width
NK = 1000            # number of 441-sample chunks per row
T = 999              # frames per row
NB = 1024            # matmul bins (bins 0..1023; Nyquist packed into sin col 0)
NMEL = 60
ROWS = 2             # batch rows per core
EPS = 1e-10
SQRT2 = math.sqrt(2.0)
PSC = 1.0 / 16.0     # power scale factor: power stored as (PSC*X)^2 = power/256
LNOFF = math.log(1.0 / (PSC * PSC))  # ln 256, the log-mel offset to repay

# chunks over the NK=1000 waveform chunk-columns
ECH = [(k * 128, min(128, NK - k * 128)) for k in range(8)]
# frame chunks (PSUM free-dim <= 512 fp32)
FCH = [(0, 512), (512, 487)]


def _host_constants(mel_filters: np.ndarray):
    """Folded DFT / mel matrices (fp8) + haar matrix (fp16)."""
    npf8 = mybir.dt.np(F8)
    m = np.arange(883)
    k = np.arange(1025)
    w = np.hamming(882).astype(np.float64)
    wm = np.zeros(883)
    wm[:882] = w
    wm1 = np.zeros(883)
    wm1[1:] = w[:882]
    th_m = 2.0 * np.pi * np.outer(m, k) / 2048.0
    th_m1 = 2.0 * np.pi * np.outer(m - 1, k) / 2048.0
    C_re = wm1[:, None] * np.cos(th_m1) - 0.97 * wm[:, None] * np.cos(th_m)
    C_im = wm1[:, None] * np.sin(th_m1) - 0.97 * wm[:, None] * np.sin(th_m)
    CW = C_re[:, 0:NB].copy()                 # (883, 1024)
    SW = C_im[:, 0:NB].copy()
    SW[:, 0] = C_re[:, NB]                    # Nyquist cos column in sin col 0

    def pack_dr(M):                           # (883,1024) -> (384, 2048)
        A = np.zeros((3, 128, 2, NB))
        for q in range(3):
            for o in range(2):
                j0 = 128 * q + 441 * o
                A[q, :, o, :] = M[j0:j0 + 128, :]
        return A.reshape(384, 2 * NB).astype(npf8)

    def pack_w4(M):                           # tail rows -> (128, 1024)
        A = np.zeros((128, NB))
        A[0:57] = M[384:441]
        A[64:122] = M[825:883]
        return A.astype(npf8)

    wdrc, wdrs = pack_dr(CW), pack_dr(SW)
    w4c, w4s = pack_w4(CW), pack_w4(SW)

    mf = mel_filters.astype(np.float64)       # (60, 1025)
    mat = mf[:, 0:NB].T                       # (1024, 60) cos-bin mel rows
    mbt = np.concatenate([mf[:, NB:NB + 1], mf[:, 1:NB]], axis=1).T
    # pad mel cols to 64 so the DoubleRow pair-dim step is 16B-aligned
    melw = np.zeros((8, 128, 2, 64))
    for bc in range(8):
        melw[bc, :, 0, 0:NMEL] = mat[bc * 128:(bc + 1) * 128, :]
        melw[bc, :, 1, 0:NMEL] = mbt[bc * 128:(bc + 1) * 128, :]
    melw = melw.reshape(1024, 128).astype(npf8)

    idn = np.eye(128, dtype=np.float16)
    # [:, :30] haar-sum; [:, 32:62] haar-diff (base-32 aligned for PSUM reads)
    hsd = np.zeros((NMEL, 64), np.float16)
    for i in range(30):
        hsd[2 * i, i] = 1.0
        hsd[2 * i + 1, i] = 1.0
        hsd[2 * i, 32 + i] = 1.0
        hsd[2 * i + 1, 32 + i] = -1.0
    return wdrc, wdrs, w4c, w4s, melw, idn, hsd


def _body(ctx: ExitStack, tc, xpad, wdrc_d, wdrs_d, w4c_d, w4s_d, melw_d,
          idn_d, hsd_d, out_d):
    nc = tc.nc

    const = ctx.enter_context(tc.tile_pool(name="const", bufs=1))
    e2p = ctx.enter_context(tc.tile_pool(name="e2", bufs=3))
    emp = ctx.enter_context(tc.tile_pool(name="em", bufs=3))
    etp = ctx.enter_context(tc.tile_pool(name="et", bufs=1))
    ptrp = ctx.enter_context(tc.tile_pool(name="ptr", bufs=2, space="PSUM"))
    dftp = ctx.enter_context(tc.tile_pool(name="dft", bufs=2, space="PSUM"))
    melp = ctx.enter_context(tc.tile_pool(name="mel", bufs=1, space="PSUM"))
    haarp = ctx.enter_context(tc.tile_pool(name="haar", bufs=1, space="PSUM"))
    ppp = ctx.enter_context(tc.tile_pool(name="pp", bufs=2))
    lmp = ctx.enter_context(tc.tile_pool(name="lm", bufs=1))
    hop = ctx.enter_context(tc.tile_pool(name="ho", bufs=1))
    stp = ctx.enter_context(tc.tile_pool(name="st", bufs=2))

    # ---- constants into SBUF
    wdrc_t, wdrs_t, melw_t = [], [], []
    for q in range(3):
        t = const.tile([128, 2, NB], F8, tag=f"wc{q}", name=f"wc{q}")
        nc.sync.dma_start(t, wdrc_d[q * 128:(q + 1) * 128, :])
        wdrc_t.append(t)
        t = const.tile([128, 2, NB], F8, tag=f"ws{q}", name=f"ws{q}")
        nc.sync.dma_start(t, wdrs_d[q * 128:(q + 1) * 128, :])
        wdrs_t.append(t)
    w4c_t = const.tile([128, NB], F8, tag="w4c", name="w4c_t")
    nc.sync.dma_start(w4c_t, w4c_d[:, :])
    w4s_t = const.tile([128, NB], F8, tag="w4s", name="w4s_t")
    nc.sync.dma_start(w4s_t, w4s_d[:, :])
    for bc in range(8):
        t = const.tile([128, 2, 64], F8, tag=f"mw{bc}", name=f"mw{bc}")
        nc.sync.dma_start(t, melw_d[bc * 128:(bc + 1) * 128, :])
        melw_t.append(t)
    ident = const.tile([128, 128], F16, tag="id", name="ident")
    nc.sync.dma_start(ident[:, :], idn_d[:, :])
    hsd_t = const.tile([NMEL, 64], F16, tag="hsd", name="hsd_t")
    nc.sync.dma_start(hsd_t[:, :], hsd_d[:, :])
    eps_t = const.tile([128, 1], F32, tag="eps", name="eps")
    nc.vector.memset(eps_t[:, :], EPS * PSC * PSC)
    lnoff_t = const.tile([30, 1], F32, tag="lnoff", name="lnoff")
    nc.vector.memset(lnoff_t[:, :], SQRT2 * LNOFF)

    for r in range(ROWS):
        et = [etp.tile([128, 2, NK], F8, tag=f"et{r}_{q}", name=f"et{r}_{q}")
              for q in range(3)]
        et.append(etp.tile([128, NK], F8, tag=f"et{r}_3", name=f"et{r}_3"))
        # tail-tile pad partitions (weights there are zero, but uninit SBUF
        # could hold NaN patterns; 0 * NaN would poison the accumulation)
        nc.vector.memset(et[3][:, :], 0.0)

        def chunk(k0, ksz):
            e2 = e2p.tile([128, W + 1], F32, tag="e2", name="e2")
            src = bass.AP(xpad, r * (L + 1) + W * k0, [[W, ksz], [1, W + 1]])
            nc.sync.dma_start(e2[0:ksz, :], src)
            em = emp.tile([128, W + 1], F16, tag="em", name="em")
            nc.vector.tensor_copy(em[0:ksz, :], e2[0:ksz, :])
            for ti, c0 in enumerate((0, 128, 256)):
                ptr = ptrp.tile([128, 128], F16, tag="ptr", name="ptr")
                nc.tensor.transpose(ptr[0:128, 0:ksz], em[0:ksz, c0:c0 + 128],
                                    ident[0:ksz, 0:ksz])
                nc.scalar.copy(et[ti][0:128, 0, k0:k0 + ksz], ptr[0:128, 0:ksz])
                if k0 == 0:
                    nc.scalar.copy(et[ti][0:128, 1, 0:ksz - 1], ptr[0:128, 1:ksz])
                else:
                    nc.scalar.copy(et[ti][0:128, 1, k0 - 1:k0 - 1 + ksz],
                                   ptr[0:128, 0:ksz])
            ptr = ptrp.tile([128, 128], F16, tag="ptr", name="ptr")
            nc.tensor.transpose(ptr[0:57, 0:ksz], em[0:ksz, 384:441],
                                ident[0:ksz, 0:ksz])
            nc.scalar.copy(et[3][0:57, k0:k0 + ksz], ptr[0:57, 0:ksz])
            ptr2 = ptrp.tile([128, 128], F16, tag="ptr", name="ptr2")
            nc.tensor.transpose(ptr2[64:122, 0:ksz], em[0:ksz, 384:442],
                                ident[0:ksz, 0:ksz])
            if k0 == 0:
                nc.scalar.copy(et[3][64:122, 0:ksz - 1], ptr2[64:122, 1:ksz])
            else:
                nc.scalar.copy(et[3][64:122, k0 - 1:k0 - 1 + ksz],
                               ptr2[64:122, 0:ksz])

        lm = lmp.tile([NMEL, T], F16, tag=f"lm{r}", name=f"lm{r}")

        def dft_fch(f0, fN):
            mp = melp.tile([NMEL, 512], F32, tag="mp", name="mp")
            for bc in range(8):
                pre = dftp.tile([128, 512], F32, tag="pre", name="pre")
                pim = dftp.tile([128, 512], F32, tag="pim", name="pim")
                for dst, wdr, w4 in ((pre, wdrc_t, w4c_t), (pim, wdrs_t, w4s_t)):
                    for q in range(3):
                        nc.tensor.matmul(dst[:, 0:fN],
                                         wdr[q][:, :, bc * 128:(bc + 1) * 128],
                                         et[q][:, 0:2, f0:f0 + fN],
                                         start=(q == 0), stop=False,
                                         perf_mode=DR)
                    nc.tensor.matmul(dst[:, 0:fN],
                                     w4[0:122, bc * 128:(bc + 1) * 128],
                                     et[3][0:122, f0:f0 + fN],
                                     start=False, stop=True)
                pa = ppp.tile([128, 2, 512], F8, tag="pa", name="pa")
                nc.scalar.activation(pa[:, 0, 0:fN], pre[:, 0:fN], AF.Square,
                                     scale=PSC)
                nc.scalar.activation(pa[:, 1, 0:fN], pim[:, 0:fN], AF.Square,
                                     scale=PSC)
                nc.tensor.matmul(mp[0:NMEL, 0:fN], melw_t[bc][:, :, 0:NMEL],
                                 pa[:, 0:2, 0:fN], start=(bc == 0),
                                 stop=(bc == 7), perf_mode=DR,
                                 skip_group_check=True)
            nc.scalar.activation(lm[0:NMEL, f0:f0 + fN], mp[0:NMEL, 0:fN],
                                 AF.Ln, bias=eps_t[0:NMEL, :])

        # interleave: transposes for the first 5 chunks, first frame block,
        # remaining chunks, second frame block
        for (k0, ksz) in ECH[:5]:
            chunk(k0, ksz)
        dft_fch(*FCH[0])
        for (k0, ksz) in ECH[5:]:
            chunk(k0, ksz)
        dft_fch(*FCH[1])

        # ---- Haar (one matmul: rows 0..29 = sum, 30..59 = diff) / delta / stats
        ca = hop.tile([30, T], F32, tag=f"ca{r}", name=f"ca{r}")
        cd = hop.tile([30, T], F32, tag=f"cd{r}", name=f"cd{r}")
        for (f0, fN) in FCH:
            po = haarp.tile([64, 512], F32, tag="po", name="po")
            nc.tensor.matmul(po[0:64, 0:fN], hsd_t[:, :], lm[0:NMEL, f0:f0 + fN],
                             start=True, stop=True, skip_group_check=True)
            nc.scalar.copy(ca[:, f0:f0 + fN], po[0:30, 0:fN])
            nc.scalar.copy(cd[:, f0:f0 + fN], po[32:62, 0:fN])
        dl = hop.tile([30, T], F32, tag=f"dl{r}", name=f"dl{r}")
        nc.vector.tensor_sub(dl[:, 1:T - 1], ca[:, 2:T], ca[:, 0:T - 2])
        nc.vector.tensor_sub(dl[:, 0:1], ca[:, 1:2], ca[:, 0:1])
        nc.vector.tensor_sub(dl[:, T - 1:T], ca[:, T - 1:T], ca[:, T - 2:T - 1])

        stats = stp.tile([30, 6], F32, tag=f"stats{r}", name=f"stats{r}")
        for si, feat in enumerate((ca, dl, cd)):
            s1 = stp.tile([30, 1], F32, tag="s1", name="s1")
            nc.vector.tensor_reduce(s1[:, :], feat[:, :], axis=mybir.AxisListType.X,
                                    op=mybir.AluOpType.add)
            if si == 0:
                # repay the ln(256) power-scale offset: mean(cA) shifted by
                # -sqrt(2)*ln(256); delta and cD differences cancel it
                nc.scalar.activation(stats[:, 0:1], s1[:, :], AF.Identity,
                                     bias=lnoff_t[:, :], scale=1.0 / (T * SQRT2))
            else:
                nc.vector.tensor_scalar_mul(stats[:, si:si + 1], s1[:, :],
                                            1.0 / (T * SQRT2))
            nm = stp.tile([30, 1], F32, tag="nm", name="nm")
            nc.vector.tensor_scalar_mul(nm[:, :], s1[:, :], -1.0 / T)
            scr = stp.tile([30, T], F32, tag="scr", name="scr")
            sq = stp.tile([30, 1], F32, tag="sq", name="sq")
            nc.scalar.activation(scr[:, :], feat[:, :], AF.Square, bias=nm[:, :],
                                 scale=1.0, accum_out=sq[:, :])
            nc.scalar.activation(stats[:, 3 + si:4 + si], sq[:, :], AF.Sqrt,
                                 scale=1.0 / ((T - 1) * 2.0))
        nc.sync.dma_start(bass.AP(out_d, r * 180, [[1, 180]]), stats[:, :])


_CACHE = {}


def _build():
    if "nc" in _CACHE:
        return _CACHE["nc"]
    nc = bacc.Bacc("TRN2", target_bir_lowering=False, debug=False,
                   enable_asserts=False, num_devices=8)
    xpad = nc.dram_tensor("xpad", [ROWS, L + 1], F32, kind="ExternalInput")
    wdrc_d = nc.dram_tensor("wdrc", [384, 2 * NB], F8, kind="ExternalInput")
    wdrs_d = nc.dram_tensor("wdrs", [384, 2 * NB], F8, kind="ExternalInput")
    w4c_d = nc.dram_tensor("w4c", [128, NB], F8, kind="ExternalInput")
    w4s_d = nc.dram_tensor("w4s", [128, NB], F8, kind="ExternalInput")
    melw_d = nc.dram_tensor("melw", [1024, 128], F8, kind="ExternalInput")
    idn_d = nc.dram_tensor("idn", [128, 128], F16, kind="ExternalInput")
    hsd_d = nc.dram_tensor("hsd", [NMEL, 64], F16, kind="ExternalInput")
    out_d = nc.dram_tensor("out", [ROWS, 180], F32, kind="ExternalOutput")
    with tile.TileContext(nc) as tc, ExitStack() as ctx:
        _body(ctx, tc, xpad, wdrc_d, wdrs_d, w4c_d, w4s_d, melw_d,
              idn_d, hsd_d, out_d)
    nc.compile()
    _CACHE["nc"] = nc
    return nc


def make_in_maps(waveform: np.ndarray, mel_filters: np.ndarray):
    wdrc, wdrs, w4c, w4s, melw, idn, hsd = _host_constants(mel_filters)
    in_maps = []
    for core in range(8):
        rows = waveform[ROWS * core:ROWS * (core + 1)]
        xp = np.zeros((ROWS, L + 1), np.float32)
        xp[:, 1:] = rows
        in_maps.append({"xpad": xp, "wdrc": wdrc, "wdrs": wdrs, "w4c": w4c,
                        "w4s": w4s, "melw": melw, "idn": idn, "hsd": hsd})
    return in_maps


def gather_out(results):
    # device rows are packed [mel_idx, stat]; reorder to [stat, mel_idx]
    full = np.concatenate([results[c]["out"] for c in range(8)], axis=0)
    return np.ascontiguousarray(
        full.reshape(B, 30, 6).transpose(0, 2, 1).reshape(B, 180)).astype(np.float32)


def run(waveform, mel_filters, trace=False):
    nc = _build()
    in_maps = make_in_maps(np.asarray(waveform, np.float32),
                           np.asarray(mel_filters, np.float32))
    res = run_bass_kernel_spmd(nc, in_maps, core_ids=list(range(8)), trace=trace)
    return gather_out(res.results), res


def kernel(waveform: np.ndarray, mel_filters: np.ndarray) -> np.ndarray:
    out, _ = run(waveform, mel_filters, trace=False)
    return out


# revision 11
# speedup vs baseline: 1.7596x; 1.1464x over previous
"""MFDWC feature extractor as a Bass/Tile kernel for TRN2 (8 NeuronCores).

v2: fp8 DoubleRow DFT with pre-emphasis folded into the DFT matrices.

Pipeline (per batch row): [pre-emphasis folded into weights] -> framing
(999 frames x 883 samples of the zero-padded raw waveform, hop 441) ->
rFFT(2048) power spectrum -> mel (60) -> log -> Haar DWT -> delta ->
mean/std over time -> 180 features.

Device mapping / math:
  - Data parallel: 16 batch rows -> 2 rows per core on 8 cores.
  - Pre-emphasis y[t] = x[t] - 0.97 x[t-1] is an LTI op, so it is folded
    into the DFT coefficient matrix: frame DFT = sum_m xpad[441 t + m] *
    C[m, k] over m in [0, 883), with C[m,k] = w[m-1] e^{-i th (m-1)} -
    0.97 w[m] e^{-i th m}. The kernel contracts 883 raw samples per frame.
  - The waveform is transposed on-chip to (441-sample chunk rows x 1000
    chunk cols) fp8 tiles; frames are overlapping column views.
  - DoubleRow fp8 matmuls contract 256 rows per instruction: pair o=0 ->
    sample j = 128 q + p (column c), o=1 -> j = 441 + 128 q + p (column
    c+1, expressed as an overlapping access pattern on the same tile).
    Tail rows j in [384,441) u [825,883) live in one combined tile
    (partitions 0..56 and 64..121, the latter written by a base-64
    tile_position transpose) contracted by one normal fp8 matmul.
  - Bins packing: cos matmul covers bins 0..1023; the sin matrix's bin-0
    column carries the Nyquist cos column; mel matrices match.
  - power -> mel is one DoubleRow matmul per 128-bin block (cos^2 block
    paired with sin^2 block); power is stored fp8 scaled by 1/256, the
    log offset is repaid in the cA-mean statistics.
"""

import math
from contextlib import ExitStack

import numpy as np

import concourse.bass as bass
import concourse.bacc as bacc
import concourse.mybir as mybir
import concourse.tile as tile
from concourse.bass_utils import run_bass_kernel_spmd

F32 = mybir.dt.float32
F16 = mybir.dt.float16
F8 = mybir.dt.float8e4
AF = mybir.ActivationFunctionType
DR = mybir.MatmulPerfMode.DoubleRow

B = 16               # batch
L = 441000           # samples per row
W = 441              # hop; also chunk width
NK = 1000            # number of 441-sample chunks per row
T = 999              # frames per row
NB = 1024            # matmul bins (bins 0..1023; Nyquist packed into sin col 0)
NMEL = 60
ROWS = 2             # batch rows per core
EPS = 1e-10
SQRT2 = math.sqrt(2.0)
PSC = 1.0 / 16.0     # power scale factor: power stored as (PSC*X)^2 = power/256
LNOFF = math.log(1.0 / (PSC * PSC))  # ln 256, the log-mel offset to repay

# chunks over the NK=1000 waveform chunk-columns
ECH = [(k * 128, min(128, NK - k * 128)) for k in range(8)]
# frame chunks (PSUM free-dim <= 512 fp32)
FCH = [(0, 512), (512, 487)]


def _host_constants(mel_filters: np.ndarray):
    """Folded DFT / mel matrices (fp8) + haar matrix (fp16)."""
    npf8 = mybir.dt.np(F8)
    m = np.arange(883)
    k = np.arange(1025)
    w = np.hamming(882).astype(np.float64)
    wm = np.zeros(883)
    wm[:882] = w
    wm1 = np.zeros(883)
    wm1[1:] = w[:882]
    th_m = 2.0 * np.pi * np.outer(m, k) / 2048.0
    th_m1 = 2.0 * np.pi * np.outer(m - 1, k) / 2048.0
    C_re = wm1[:, None] * np.cos(th_m1) - 0.97 * wm[:, None] * np.cos(th_m)
    C_im = wm1[:, None] * np.sin(th_m1) - 0.97 * wm[:, None] * np.sin(th_m)
    CW = C_re[:, 0:NB].copy()                 # (883, 1024)
    SW = C_im[:, 0:NB].copy()
    SW[:, 0] = C_re[:, NB]                    # Nyquist cos column in sin col 0

    def pack_dr(M):                           # (883,1024) -> (384, 2048)
        A = np.zeros((3, 128, 2, NB))
        for q in range(3):
            for o in range(2):
                j0 = 128 * q + 441 * o
                A[q, :, o, :] = M[j0:j0 + 128, :]
        return A.reshape(384, 2 * NB).astype(npf8)

    def pack_w4(M):                           # tail rows -> (128, 1024)
        A = np.zeros((128, NB))
        A[0:57] = M[384:441]
        A[64:122] = M[825:883]
        return A.astype(npf8)

    wdrc, wdrs = pack_dr(CW), pack_dr(SW)
    w4c, w4s = pack_w4(CW), pack_w4(SW)

    mf = mel_filters.astype(np.float64)       # (60, 1025)
    mat = mf[:, 0:NB].T                       # (1024, 60) cos-bin mel rows
    mbt = np.concatenate([mf[:, NB:NB + 1], mf[:, 1:NB]], axis=1).T
    # pad mel cols to 64 so the DoubleRow pair-dim step is 16B-aligned
    melw = np.zeros((8, 128, 2, 64))
    for bc in range(8):
        melw[bc, :, 0, 0:NMEL] = mat[bc * 128:(bc + 1) * 128, :]
        melw[bc, :, 1, 0:NMEL] = mbt[bc * 128:(bc + 1) * 128, :]
    melw = melw.reshape(1024, 128).astype(npf8)

    idn = np.eye(128, dtype=np.float16)
    # [:, :30] haar-sum; [:, 32:62] haar-diff (base-32 aligned for PSUM reads)
    hsd = np.zeros((NMEL, 64), np.float16)
    for i in range(30):
        hsd[2 * i, i] = 1.0
        hsd[2 * i + 1, i] = 1.0
        hsd[2 * i, 32 + i] = 1.0
        hsd[2 * i + 1, 32 + i] = -1.0
    return wdrc, wdrs, w4c, w4s, melw, idn, hsd


def _body(ctx: ExitStack, tc, xpad, wdrc_d, wdrs_d, w4c_d, w4s_d, melw_d,
          idn_d, hsd_d, out_d):
    nc = tc.nc

    const = ctx.enter_context(tc.tile_pool(name="const", bufs=1))
    e2p = ctx.enter_context(tc.tile_pool(name="e2", bufs=3))
    emp = ctx.enter_context(tc.tile_pool(name="em", bufs=3))
    etp = ctx.enter_context(tc.tile_pool(name="et", bufs=1))
    ptrp = ctx.enter_context(tc.tile_pool(name="ptr", bufs=2, space="PSUM"))
    dftp = ctx.enter_context(tc.tile_pool(name="dft", bufs=2, space="PSUM"))
    melp = ctx.enter_context(tc.tile_pool(name="mel", bufs=1, space="PSUM"))
    haarp = ctx.enter_context(tc.tile_pool(name="haar", bufs=1, space="PSUM"))
    ppp = ctx.enter_context(tc.tile_pool(name="pp", bufs=2))
    lmp = ctx.enter_context(tc.tile_pool(name="lm", bufs=1))
    hop = ctx.enter_context(tc.tile_pool(name="ho", bufs=1))
    stp = ctx.enter_context(tc.tile_pool(name="st", bufs=2))

    # ---- constants into SBUF
    ident = const.tile([128, 128], F16, tag="id", name="ident")
    nc.sync.dma_start(ident[:, :], idn_d[:, :])
    # PE warmup: ~4us of dense matmuls flips the HAM clock gate to 8/8
    # before the real work starts (transpose-mode does not count as
    # PE-activity for the governor)
    wu = dftp.tile([128, 512], F32, tag="pre", name="wu")
    for _ in range(36):
        nc.tensor.matmul(wu[:, 0:128], ident[:, :], ident[:, :],
                         start=True, stop=True, skip_group_check=True)
    wdrc_t, wdrs_t, melw_t = [], [], []
    for q in range(3):
        t = const.tile([128, 2, NB], F8, tag=f"wc{q}", name=f"wc{q}")
        nc.sync.dma_start(t, wdrc_d[q * 128:(q + 1) * 128, :])
        wdrc_t.append(t)
        t = const.tile([128, 2, NB], F8, tag=f"ws{q}", name=f"ws{q}")
        nc.sync.dma_start(t, wdrs_d[q * 128:(q + 1) * 128, :])
        wdrs_t.append(t)
    w4c_t = const.tile([128, NB], F8, tag="w4c", name="w4c_t")
    nc.sync.dma_start(w4c_t, w4c_d[:, :])
    w4s_t = const.tile([128, NB], F8, tag="w4s", name="w4s_t")
    nc.sync.dma_start(w4s_t, w4s_d[:, :])
    for bc in range(8):
        t = const.tile([128, 2, 64], F8, tag=f"mw{bc}", name=f"mw{bc}")
        nc.sync.dma_start(t, melw_d[bc * 128:(bc + 1) * 128, :])
        melw_t.append(t)
    hsd_t = const.tile([NMEL, 64], F16, tag="hsd", name="hsd_t")
    nc.sync.dma_start(hsd_t[:, :], hsd_d[:, :])
    eps_t = const.tile([128, 1], F32, tag="eps", name="eps")
    nc.vector.memset(eps_t[:, :], EPS * PSC * PSC)
    lnoff_t = const.tile([30, 1], F32, tag="lnoff", name="lnoff")
    nc.vector.memset(lnoff_t[:, :], SQRT2 * LNOFF)

    for r in range(ROWS):
        et = [etp.tile([128, 2, NK], F8, tag=f"et{r}_{q}", name=f"et{r}_{q}")
              for q in range(3)]
        et.append(etp.tile([128, NK], F8, tag=f"et{r}_3", name=f"et{r}_3"))
        # tail-tile pad partitions (weights there are zero, but uninit SBUF
        # could hold NaN patterns; 0 * NaN would poison the accumulation)
        nc.vector.memset(et[3][:, :], 0.0)

        def chunk(k0, ksz):
            e2 = e2p.tile([128, W + 1], F32, tag="e2", name="e2")
            src = bass.AP(xpad, r * (L + 1) + W * k0, [[W, ksz], [1, W + 1]])
            nc.sync.dma_start(e2[0:ksz, :], src)
            em = emp.tile([128, W + 1], F16, tag="em", name="em")
            nc.vector.tensor_copy(em[0:ksz, :], e2[0:ksz, :])
            for ti, c0 in enumerate((0, 128, 256)):
                ptr = ptrp.tile([128, 128], F16, tag="ptr", name="ptr")
                nc.tensor.transpose(ptr[0:128, 0:ksz], em[0:ksz, c0:c0 + 128],
                                    ident[0:ksz, 0:ksz])
                if k0 == 0:
                    nc.vector.tensor_copy(et[ti][0:128, 0, 0:ksz], ptr[0:128, 0:ksz])
                    nc.vector.tensor_copy(et[ti][0:128, 1, 0:ksz - 1],
                                          ptr[0:128, 1:ksz])
                else:
                    pitch = et[ti].ap[0][0]
                    ostride = et[ti].ap[1][0]
                    dst = bass.AP(tensor=et[ti].tensor,
                                  offset=et[ti].offset + k0,
                                  ap=[[pitch, 128], [ostride - 1, 2], [1, ksz]])
                    ppitch = ptr.ap[0][0]
                    sb = bass.AP(tensor=ptr.tensor, offset=ptr.offset,
                                 ap=[[ppitch, 128], [0, 2], [1, ksz]])
                    nc.vector.tensor_copy(dst, sb)
            ptr = ptrp.tile([128, 128], F16, tag="ptr", name="ptr")
            nc.tensor.transpose(ptr[0:57, 0:ksz], em[0:ksz, 384:441],
                                ident[0:ksz, 0:ksz])
            nc.scalar.copy(et[3][0:57, k0:k0 + ksz], ptr[0:57, 0:ksz])
            ptr2 = ptrp.tile([128, 128], F16, tag="ptr", name="ptr2")
            nc.tensor.transpose(ptr2[64:122, 0:ksz], em[0:ksz, 384:442],
                                ident[0:ksz, 0:ksz])
            if k0 == 0:
                nc.scalar.copy(et[3][64:122, 0:ksz - 1], ptr2[64:122, 1:ksz])
            else:
                nc.scalar.copy(et[3][64:122, k0 - 1:k0 - 1 + ksz],
                               ptr2[64:122, 0:ksz])

        lm = lmp.tile([NMEL, T], F16, tag=f"lm{r}", name=f"lm{r}")

        def dft_fch(f0, fN, inter=None):
            inter = inter or {}
            mp = melp.tile([NMEL, 512], F32, tag="mp", name="mp")
            for bc in range(8):
                if bc in inter:
                    chunk(*inter[bc])
                pre = dftp.tile([128, 512], F32, tag="pre", name="pre")
                pim = dftp.tile([128, 512], F32, tag="pim", name="pim")
                for dst, wdr, w4 in ((pre, wdrc_t, w4c_t), (pim, wdrs_t, w4s_t)):
                    for q in range(3):
                        nc.tensor.matmul(dst[:, 0:fN],
                                         wdr[q][:, :, bc * 128:(bc + 1) * 128],
                                         et[q][:, 0:2, f0:f0 + fN],
                                         start=(q == 0), stop=False,
                                         perf_mode=DR)
                    nc.tensor.matmul(dst[:, 0:fN],
                                     w4[0:122, bc * 128:(bc + 1) * 128],
                                     et[3][0:122, f0:f0 + fN],
                                     start=False, stop=True)
                pa = ppp.tile([128, 2, 512], F8, tag="pa", name="pa")
                nc.scalar.activation(pa[:, 0, 0:fN], pre[:, 0:fN], AF.Square,
                                     scale=PSC)
                nc.scalar.activation(pa[:, 1, 0:fN], pim[:, 0:fN], AF.Square,
                                     scale=PSC)
                nc.tensor.matmul(mp[0:NMEL, 0:fN], melw_t[bc][:, :, 0:NMEL],
                                 pa[:, 0:2, 0:fN], start=(bc == 0),
                                 stop=(bc == 7), perf_mode=DR,
                                 skip_group_check=True)
            nc.scalar.activation(lm[0:NMEL, f0:f0 + fN], mp[0:NMEL, 0:fN],
                                 AF.Ln, bias=eps_t[0:NMEL, :])

        # transposes for the first 5 chunks feed the first frame block; the
        # remaining chunks are interleaved between its bc groups so the PE
        # never has a transpose-only stretch (keeps the HAM clock warm)
        for (k0, ksz) in ECH[:5]:
            chunk(k0, ksz)
        dft_fch(*FCH[0], inter={2: ECH[5], 4: ECH[6], 6: ECH[7]})
        dft_fch(*FCH[1])

        # ---- Haar (one matmul: rows 0..29 = sum, 30..59 = diff) / delta / stats
        ca = hop.tile([30, T], F32, tag=f"ca{r}", name=f"ca{r}")
        cd = hop.tile([30, T], F32, tag=f"cd{r}", name=f"cd{r}")
        for (f0, fN) in FCH:
            po = haarp.tile([64, 512], F32, tag="po", name="po")
            nc.tensor.matmul(po[0:64, 0:fN], hsd_t[:, :], lm[0:NMEL, f0:f0 + fN],
                             start=True, stop=True, skip_group_check=True)
            nc.scalar.copy(ca[:, f0:f0 + fN], po[0:30, 0:fN])
            nc.scalar.copy(cd[:, f0:f0 + fN], po[32:62, 0:fN])
        dl = hop.tile([30, T], F32, tag=f"dl{r}", name=f"dl{r}")
        nc.vector.tensor_sub(dl[:, 1:T - 1], ca[:, 2:T], ca[:, 0:T - 2])
        nc.vector.tensor_sub(dl[:, 0:1], ca[:, 1:2], ca[:, 0:1])
        nc.vector.tensor_sub(dl[:, T - 1:T], ca[:, T - 1:T], ca[:, T - 2:T - 1])

        stats = stp.tile([30, 6], F32, tag=f"stats{r}", name=f"stats{r}")
        for si, feat in enumerate((ca, dl, cd)):
            s1 = stp.tile([30, 1], F32, tag="s1", name="s1")
            nc.vector.tensor_reduce(s1[:, :], feat[:, :], axis=mybir.AxisListType.X,
                                    op=mybir.AluOpType.add)
            if si == 0:
                # repay the ln(256) power-scale offset: mean(cA) shifted by
                # -sqrt(2)*ln(256); delta and cD differences cancel it
                nc.scalar.activation(stats[:, 0:1], s1[:, :], AF.Identity,
                                     bias=lnoff_t[:, :], scale=1.0 / (T * SQRT2))
            else:
                nc.vector.tensor_scalar_mul(stats[:, si:si + 1], s1[:, :],
                                            1.0 / (T * SQRT2))
            nm = stp.tile([30, 1], F32, tag="nm", name="nm")
            nc.vector.tensor_scalar_mul(nm[:, :], s1[:, :], -1.0 / T)
            scr = stp.tile([30, T], F32, tag="scr", name="scr")
            sq = stp.tile([30, 1], F32, tag="sq", name="sq")
            nc.scalar.activation(scr[:, :], feat[:, :], AF.Square, bias=nm[:, :],
                                 scale=1.0, accum_out=sq[:, :])
            nc.scalar.activation(stats[:, 3 + si:4 + si], sq[:, :], AF.Sqrt,
                                 scale=1.0 / ((T - 1) * 2.0))
        nc.sync.dma_start(bass.AP(out_d, r * 180, [[1, 180]]), stats[:, :])


_CACHE = {}


def _build():
    if "nc" in _CACHE:
        return _CACHE["nc"]
    nc = bacc.Bacc("TRN2", target_bir_lowering=False, debug=False,
                   enable_asserts=False, num_devices=8)
    xpad = nc.dram_tensor("xpad", [ROWS, L + 1], F32, kind="ExternalInput")
    wdrc_d = nc.dram_tensor("wdrc", [384, 2 * NB], F8, kind="ExternalInput")
    wdrs_d = nc.dram_tensor("wdrs", [384, 2 * NB], F8, kind="ExternalInput")
    w4c_d = nc.dram_tensor("w4c", [128, NB], F8, kind="ExternalInput")
    w4s_d = nc.dram_tensor("w4s", [128, NB], F8, kind="ExternalInput")
    melw_d = nc.dram_tensor("melw", [1024, 128], F8, kind="ExternalInput")
    idn_d = nc.dram_tensor("idn", [128, 128], F16, kind="ExternalInput")
    hsd_d = nc.dram_tensor("hsd", [NMEL, 64], F16, kind="ExternalInput")
    out_d = nc.dram_tensor("out", [ROWS, 180], F32, kind="ExternalOutput")
    with tile.TileContext(nc) as tc, ExitStack() as ctx:
        _body(ctx, tc, xpad, wdrc_d, wdrs_d, w4c_d, w4s_d, melw_d,
              idn_d, hsd_d, out_d)
    nc.compile()
    _CACHE["nc"] = nc
    return nc


def make_in_maps(waveform: np.ndarray, mel_filters: np.ndarray):
    wdrc, wdrs, w4c, w4s, melw, idn, hsd = _host_constants(mel_filters)
    in_maps = []
    for core in range(8):
        rows = waveform[ROWS * core:ROWS * (core + 1)]
        xp = np.zeros((ROWS, L + 1), np.float32)
        xp[:, 1:] = rows
        in_maps.append({"xpad": xp, "wdrc": wdrc, "wdrs": wdrs, "w4c": w4c,
                        "w4s": w4s, "melw": melw, "idn": idn, "hsd": hsd})
    return in_maps


def gather_out(results):
    # device rows are packed [mel_idx, stat]; reorder to [stat, mel_idx]
    full = np.concatenate([results[c]["out"] for c in range(8)], axis=0)
    return np.ascontiguousarray(
        full.reshape(B, 30, 6).transpose(0, 2, 1).reshape(B, 180)).astype(np.float32)


def run(waveform, mel_filters, trace=False):
    nc = _build()
    in_maps = make_in_maps(np.asarray(waveform, np.float32),
                           np.asarray(mel_filters, np.float32))
    res = run_bass_kernel_spmd(nc, in_maps, core_ids=list(range(8)), trace=trace)
    return gather_out(res.results), res


def kernel(waveform: np.ndarray, mel_filters: np.ndarray) -> np.ndarray:
    out, _ = run(waveform, mel_filters, trace=False)
    return out
